# revision 6
# baseline (speedup 1.0000x reference)
"""Trainium2 Bass kernel for the CAML-style CNN + label-wise attention model.

Strategy: pure batch data-parallelism across the 8 NeuronCores (one batch
element per core, weights replicated, no cross-device communication).  Each
core computes, for its batch element b:

    emb   = embed_W[tokens[b]]                  (indirect-DMA gather)
    hT    = tanh(conv1d(embT) + b)              [F, L]   (PE matmuls, 9 taps)
    h_aug = [h | 1]                             [L, F+1] (PE transposes of hT)
    attT  = hT.T @ U_w.T                        [L, Y]   (PE)
    eT    = exp(attT)                           (ACT, single exp pass)
    m_aug = eT.T @ h_aug  -> m_unnorm | s       [Y, F+1] (PE; s = softmax denom)
    m     = m_unnorm / s                        (DVE)
    alphaT= eT * (1/s)                          (DVE+GPSIMD split)
    logit = sum_f m * final_w                   (DVE fused mul+reduce)

The full outputs are assembled on the host (stack over batch, transpose
alphaT -> alpha, add final_b, BCE loss reduction).
"""

import os
import sys

import numpy as np

try:
    import concourse.bass as bass  # noqa: F401
except ImportError:  # pragma: no cover
    sys.path.insert(0, "/opt/trn_rl_repo")

import ml_dtypes
from contextlib import ExitStack

import concourse.bass as bass
import concourse.mybir as mybir
import concourse.tile as tile
from concourse import bacc
from concourse.bass_utils import run_bass_kernel_spmd
from concourse.masks import make_identity

BF16 = mybir.dt.bfloat16
F32 = mybir.dt.float32
I32 = mybir.dt.int32

# Model dims (hardcoded per problem spec)
B, L, V, E, F, KT, Y = 8, 2500, 50000, 100, 256, 9, 8921
PAD = KT // 2  # 4

P = 128          # partitions
YW = 1024        # y-chunk width (2 PSUM banks)
CONV_NW = 500    # conv free-dim chunk


def _ceil_div(a, b):
    return (a + b - 1) // b


def build_program(L=L, Y=Y, n_cores=8):
    """Build the (identical-on-every-core) Bass program."""
    ltn = _ceil_div(L, P)            # number of l tiles
    lpad = ltn * P
    FA = F + 1                       # h_aug row width (ones column at F)

    nc = bacc.Bacc("TRN2", target_bir_lowering=False, debug=False,
                   num_devices=n_cores)

    # ---------------- DRAM I/O ----------------
    tok_d = nc.dram_tensor("tok", [P, ltn], I32, kind="ExternalInput").ap()
    emb_d = nc.dram_tensor("emb", [V, E], BF16, kind="ExternalInput").ap()
    uwT_d = nc.dram_tensor("uwT", [F, Y], BF16, kind="ExternalInput").ap()
    convw_d = nc.dram_tensor("convw", [KT * E, F], BF16,
                             kind="ExternalInput").ap()
    convb_d = nc.dram_tensor("convb", [F, 1], F32, kind="ExternalInput").ap()
    fw_d = nc.dram_tensor("fw", [Y, F], BF16, kind="ExternalInput").ap()

    ytn = _ceil_div(Y, P)            # number of y tiles (for logits layout)
    alphaT_d = nc.dram_tensor("alphaT", [L, Y], F32,
                              kind="ExternalOutput").ap()
    m_d = nc.dram_tensor("m_out", [Y, F], F32, kind="ExternalOutput").ap()
    logits_d = nc.dram_tensor("logits", [P, ytn], F32,
                              kind="ExternalOutput").ap()

    with tile.TileContext(nc) as tc, ExitStack() as ctx:
        # ---------------- pools ----------------
        consts = ctx.enter_context(tc.tile_pool(name="consts", bufs=1))
        gat_p = ctx.enter_context(tc.tile_pool(name="gat", bufs=4))
        pp_big = ctx.enter_context(tc.tile_pool(name="ppbig", bufs=2,
                                                space="PSUM"))
        pp_m = ctx.enter_context(tc.tile_pool(name="ppm", bufs=2,
                                              space="PSUM"))
        pp_t = ctx.enter_context(tc.tile_pool(name="ppt", bufs=1,
                                              space="PSUM"))
        eT_p = ctx.enter_context(tc.tile_pool(name="eTp", bufs=ltn + 4))
        asb_p = ctx.enter_context(tc.tile_pool(name="asbp", bufs=6))
        rb_p = ctx.enter_context(tc.tile_pool(name="rbp", bufs=2))
        sml_p = ctx.enter_context(tc.tile_pool(name="smlp", bufs=6))
        msb_p = ctx.enter_context(tc.tile_pool(name="msbp", bufs=3))

        # ---------------- constants / weights ----------------
        id_bf = consts.tile([P, P], BF16)
        make_identity(nc, id_bf)
        id_f32 = consts.tile([P, P], F32)
        make_identity(nc, id_f32)

        tok_sb = consts.tile([P, ltn], I32)
        nc.sync.dma_start(out=tok_sb[:, :], in_=tok_d[:, :])

        convw_sb = consts.tile([E, KT * F], BF16)
        for k in range(KT):
            nc.sync.dma_start(out=convw_sb[:, k * F:(k + 1) * F],
                              in_=convw_d[k * E:(k + 1) * E, :])
        convb0 = consts.tile([P, 1], F32)
        convb1 = consts.tile([P, 1], F32)
        nc.sync.dma_start(out=convb0[:, :], in_=convb_d[0:P, :])
        nc.sync.dma_start(out=convb1[:, :], in_=convb_d[P:2 * P, :])
        convb_t = [convb0, convb1]

        uwT0 = consts.tile([P, Y], BF16)
        uwT1 = consts.tile([P, Y], BF16)
        nc.sync.dma_start(out=uwT0[:, :], in_=uwT_d[0:P, :])
        nc.sync.dma_start(out=uwT1[:, :], in_=uwT_d[P:2 * P, :])
        uwT_t = [uwT0, uwT1]

        logits_sb = consts.tile([P, ytn], F32)
        nc.vector.memset(logits_sb[:, :], 0.0)

        # ---------------- embedding gather + transpose ----------------
        embT = consts.tile([P, L + 2 * PAD], BF16)   # rows 0..E-1 used
        nc.gpsimd.memset(embT[:, :], 0.0)

        for lt in range(ltn):
            rows = min(P, L - lt * P)
            g_t = gat_p.tile([P, E], BF16, tag="gat")
            nc.gpsimd.indirect_dma_start(
                out=g_t[:, :],
                out_offset=None,
                in_=emb_d[:, :],
                in_offset=bass.IndirectOffsetOnAxis(ap=tok_sb[:, lt:lt + 1],
                                                    axis=0),
            )
            tp = pp_t.tile([P, P], BF16, tag="pptb")
            nc.tensor.transpose(out=tp[0:E, :], in_=g_t[:, :],
                                identity=id_bf[:, :])
            nc.vector.tensor_copy(out=embT[0:E, PAD + lt * P:
                                           PAD + lt * P + rows],
                                  in_=tp[0:E, 0:rows])

        # ---------------- conv1d + tanh -> hT (bf16) ----------------
        hT0 = consts.tile([P, L], BF16)
        hT1 = consts.tile([P, L], BF16)
        hT_t = [hT0, hT1]
        ncw = _ceil_div(L, CONV_NW)
        for fc in range(2):
            for j in range(ncw):
                nw = min(CONV_NW, L - j * CONV_NW)
                cp = pp_big.tile([P, YW], F32, tag="ppbig")
                for k in range(KT):
                    nc.tensor.matmul(
                        out=cp[:, 0:nw],
                        lhsT=convw_sb[:, k * F + fc * P:k * F + fc * P + P],
                        rhs=embT[0:E, j * CONV_NW + k:j * CONV_NW + k + nw],
                        start=(k == 0), stop=(k == KT - 1),
                    )
                nc.scalar.activation(out=hT_t[fc][:, j * CONV_NW:
                                                  j * CONV_NW + nw],
                                     in_=cp[:, 0:nw],
                                     func=mybir.ActivationFunctionType.Tanh,
                                     bias=convb_t[fc][:, :], scale=1.0)

        # ---------------- h_aug [L, F+1] with ones column ----------------
        h_aug = consts.tile([P, ltn * FA], BF16)
        nc.gpsimd.memset(h_aug[:, :], 0.0)
        for lt in range(ltn):
            rows = min(P, L - lt * P)
            for fc in range(2):
                tp = pp_t.tile([P, P], BF16, tag="pptb")
                nc.tensor.transpose(out=tp[0:rows, :],
                                    in_=hT_t[fc][:, lt * P:lt * P + rows],
                                    identity=id_bf[:, :])
                nc.vector.tensor_copy(
                    out=h_aug[0:rows, lt * FA + fc * P:lt * FA + fc * P + P],
                    in_=tp[0:rows, :])
            nc.gpsimd.memset(h_aug[0:rows, lt * FA + F:lt * FA + F + 1], 1.0)

        # ---------------- main y-chunk loop ----------------
        n_chunks = _ceil_div(Y, YW)
        for ci in range(n_chunks):
            y0 = ci * YW
            yw = min(YW, Y - y0)
            nyt = _ceil_div(yw, P)

            # attT + exp
            eT_l = []
            for lt in range(ltn):
                rows = min(P, L - lt * P)
                ap = pp_big.tile([P, YW], F32, tag="ppbig")
                for kc in range(2):
                    for ns in range(_ceil_div(yw, 512)):
                        nw = min(512, yw - ns * 512)
                        nc.tensor.matmul(
                            out=ap[0:rows, ns * 512:ns * 512 + nw],
                            lhsT=hT_t[kc][:, lt * P:lt * P + rows],
                            rhs=uwT_t[kc][:, y0 + ns * 512:y0 + ns * 512 + nw],
                            start=(kc == 0), stop=(kc == 1),
                        )
                eT = eT_p.tile([P, YW], BF16, tag="eT")
                nc.scalar.activation(out=eT[0:rows, 0:yw],
                                     in_=ap[0:rows, 0:yw],
                                     func=mybir.ActivationFunctionType.Exp)
                eT_l.append(eT)

            # m_aug (+ softmax denominators), m, logits, 1/s row
            r_row = sml_p.tile([1, YW], F32, tag="rrow")
            for yt in range(nyt):
                yr = min(P, yw - yt * P)
                mp = pp_m.tile([P, FA], F32, tag="ppm")
                for lt in range(ltn):
                    rows = min(P, L - lt * P)
                    nc.tensor.matmul(
                        out=mp[0:yr, :],
                        lhsT=eT_l[lt][0:rows, yt * P:yt * P + yr],
                        rhs=h_aug[0:rows, lt * FA:lt * FA + FA],
                        start=(lt == 0), stop=(lt == ltn - 1),
                    )
                rs = sml_p.tile([P, 1], F32, tag="rs")
                nc.vector.reciprocal(rs[0:yr, :], mp[0:yr, F:F + 1])
                m_sb = msb_p.tile([P, F], F32, tag="msb")
                nc.vector.tensor_scalar_mul(m_sb[0:yr, :], mp[0:yr, 0:F],
                                            rs[0:yr, :])
                nc.sync.dma_start(out=m_d[y0 + yt * P:y0 + yt * P + yr, :],
                                  in_=m_sb[0:yr, :])
                fw_t = msb_p.tile([P, F], BF16, tag="fwt")
                nc.sync.dma_start(out=fw_t[0:yr, :],
                                  in_=fw_d[y0 + yt * P:y0 + yt * P + yr, :])
                scr = msb_p.tile([P, F], F32, tag="scr")
                nc.vector.tensor_tensor(out=scr[0:yr, :], in0=m_sb[0:yr, :],
                                        in1=fw_t[0:yr, :],
                                        op=mybir.AluOpType.mult)
                nc.vector.tensor_reduce(
                    out=logits_sb[0:yr, y0 // P + yt:y0 // P + yt + 1],
                    in_=scr[0:yr, :], axis=mybir.AxisListType.X,
                    op=mybir.AluOpType.add)
                # 1/s column -> row
                tp = pp_t.tile([P, P], F32, tag="pptf")
                nc.tensor.transpose(out=tp[0:1, 0:yr], in_=rs[0:yr, 0:1],
                                    identity=id_f32[0:yr, 0:yr])
                nc.vector.tensor_copy(out=r_row[0:1, yt * P:yt * P + yr],
                                      in_=tp[0:1, 0:yr])

            # broadcast 1/s row to all partitions
            rB = rb_p.tile([P, YW], F32, tag="rB")
            nc.gpsimd.partition_broadcast(out_ap=rB[:, 0:yw],
                                          in_ap=r_row[0:1, 0:yw])

            # alphaT = eT * (1/s), split across DVE and GPSIMD
            for lt in range(ltn):
                rows = min(P, L - lt * P)
                asb = asb_p.tile([P, YW], F32, tag="asb")
                eng = nc.vector if (lt % 2 == 0) else nc.gpsimd
                eng.tensor_tensor(out=asb[0:rows, 0:yw],
                                  in0=eT_l[lt][0:rows, 0:yw],
                                  in1=rB[0:rows, 0:yw],
                                  op=mybir.AluOpType.mult)
                nc.sync.dma_start(out=alphaT_d[lt * P:lt * P + rows,
                                               y0:y0 + yw],
                                  in_=asb[0:rows, 0:yw])

        nc.sync.dma_start(out=logits_d[:, :], in_=logits_sb[:, :])

    nc.compile()
    return nc


_PROGRAM = None


def _get_program():
    global _PROGRAM
    if _PROGRAM is None:
        _PROGRAM = build_program()
    return _PROGRAM


def _prep_in_maps(tokens, embed_W, conv_w, conv_b, U_w, final_w):
    bf = ml_dtypes.bfloat16
    ltn = _ceil_div(L, P)
    embed_bf = embed_W.astype(bf)
    uwT = np.ascontiguousarray(U_w.T).astype(bf)                 # [F, Y]
    convw = np.ascontiguousarray(
        conv_w.transpose(2, 1, 0)).reshape(KT * E, F).astype(bf)  # [K*E, F]
    convb = np.ascontiguousarray(conv_b.reshape(F, 1).astype(np.float32))
    fw = final_w.astype(bf)                                       # [Y, F]
    in_maps = []
    for b in range(B):
        tk = np.zeros(ltn * P, np.int32)
        tk[:L] = tokens[b].astype(np.int64)
        tk = np.ascontiguousarray(tk.reshape(ltn, P).T)           # [P, ltn]
        in_maps.append({"tok": tk, "emb": embed_bf, "uwT": uwT,
                        "convw": convw, "convb": convb, "fw": fw})
    return in_maps


def _execute(inputs, trace=False):
    in_maps = _prep_in_maps(
        np.asarray(inputs["tokens"]),
        np.asarray(inputs["embed_W"], dtype=np.float32),
        np.asarray(inputs["conv_w"], dtype=np.float32),
        np.asarray(inputs["conv_b"], dtype=np.float32),
        np.asarray(inputs["U_w"], dtype=np.float32),
        np.asarray(inputs["final_w"], dtype=np.float32),
    )
    nc = _get_program()
    res = run_bass_kernel_spmd(nc, in_maps, core_ids=list(range(B)),
                               trace=trace)
    return res


def _build_runner(nc, in_maps, n_cores=B):
    """Build a reusable jitted SPMD executor for the program (mirrors
    bass2jax.run_bass_via_pjrt, but returns a callable usable for repeated
    timed execution with donation chaining)."""
    import jax
    from jax.sharding import Mesh, NamedSharding, PartitionSpec
    from jax.experimental.shard_map import shard_map
    from concourse.bass2jax import (_bass_exec_p, install_neuronx_cc_hook,
                                    partition_id_tensor)

    install_neuronx_cc_hook()
    partition_name = (nc.partition_id_tensor.name
                      if nc.partition_id_tensor else None)
    in_names, out_names, out_avals, zero_shapes = [], [], [], []
    for alloc in nc.m.functions[0].allocations:
        if not isinstance(alloc, mybir.MemoryLocationSet):
            continue
        name = alloc.memorylocations[0].name
        if alloc.kind == "ExternalInput":
            if name != partition_name:
                in_names.append(name)
        elif alloc.kind == "ExternalOutput":
            out_names.append(name)
            shape = tuple(alloc.tensor_shape)
            dtype = mybir.dt.np(alloc.dtype)
            out_avals.append(jax.core.ShapedArray(shape, dtype))
            zero_shapes.append((shape, dtype))
    n_params = len(in_names)
    n_outs = len(out_avals)
    all_in_names = list(in_names) + list(out_names)
    if partition_name is not None:
        all_in_names.append(partition_name)

    donate = tuple(range(n_params, n_params + n_outs))

    def _body(*args):
        operands = list(args)
        if partition_name is not None:
            operands.append(partition_id_tensor())
        outs = _bass_exec_p.bind(
            *operands,
            out_avals=tuple(out_avals),
            in_names=tuple(all_in_names),
            out_names=tuple(out_names),
            lowering_input_output_aliases=(),
            sim_require_finite=True,
            sim_require_nnan=True,
            nc=nc,
        )
        return tuple(outs)

    devices = jax.devices()[:n_cores]
    mesh = Mesh(np.asarray(devices), ("core",))
    in_specs = (PartitionSpec("core"),) * (n_params + n_outs)
    out_specs = (PartitionSpec("core"),) * len(out_names)
    sharded = jax.jit(
        shard_map(_body, mesh=mesh, in_specs=in_specs, out_specs=out_specs,
                  check_rep=False),
        donate_argnums=donate, keep_unused=True,
    )
    sh = NamedSharding(mesh, PartitionSpec("core"))
    concat_in = [
        jax.device_put(
            np.concatenate([np.asarray(in_maps[c][nm]) for c in
                            range(n_cores)], axis=0), sh)
        for nm in in_names
    ]
    zeros = [
        jax.device_put(np.zeros((n_cores * s[0], *s[1:]), d), sh)
        for (s, d) in zero_shapes
    ]

    state = {"bufs": tuple(zeros)}

    def run():
        outs = sharded(*concat_in, *state["bufs"])
        state["bufs"] = tuple(outs)
        return outs

    def fetch(outs):
        return [
            {name: np.asarray(outs[i]).reshape(n_cores, *out_avals[i].shape)[c]
             for i, name in enumerate(out_names)}
            for c in range(n_cores)
        ]

    return run, fetch


def kernel(**inputs):
    target = np.asarray(inputs["target"], dtype=np.float32)
    final_b = np.asarray(inputs["final_b"], dtype=np.float32)
    res = _execute(inputs)

    ytn = _ceil_div(Y, P)
    logits = np.empty((B, Y), np.float32)
    alpha = np.empty((B, Y, L), np.float32)
    m = np.empty((B, Y, F), np.float32)
    for b in range(B):
        r = res.results[b]
        alpha[b] = r["alphaT"].T
        m[b] = r["m_out"]
        logits[b] = r["logits"].T.reshape(ytn * P)[:Y]
    logits += final_b[None, :]
    ll = np.logaddexp(0.0, logits.astype(np.float64)) \
        - target.astype(np.float64) * logits.astype(np.float64)
    loss = np.float32(np.mean(ll))
    return (logits, loss, alpha, m)
